# revision 7
# baseline (speedup 1.0000x reference)
"""Multi-head attention (nn_MultiHeadAttention_67207648248315) on 8 TRN2 NeuronCores.

Sharding: core i handles batch b = i//2 and head-group hg = i%2 (8 of 16 heads).
Per core, everything is computed in a transposed layout (feature/key dim on
SBUF partitions) so the PE array contracts along partitions without any
on-chip activation transposes:

  qT/kT = W{q,k}_hg @ X_b.T          [512=(h,d), 2048=s]   (f32r matmuls)
  v     = V_b @ Wv_hg.T (+ ones col) [2048=s, 8*(64+1)]
  scoresT_h = kT_h.T-chunks @ qT_h   [s2, s1] tiles, softmax across partitions:
     exp on ACT -> expT; AV matmul with [v_h | 1] piggybacks the column sums;
     reciprocal broadcast across partitions via a rank-1 ones (x) recip matmul;
     normalized expT is the attention output (written s2-major, host views it
     back as [s1, s2]).
  outT_partial = Wo_hg-slice.T @ ctxT  [1024=e, 2048=s]

Host: pre-transposes/pre-rounds inputs (float32r = fp32 with 11 mantissa
bits), sums the two head-group partials for `out`, adds biases bo, and
returns `attn` as a transposed view of the per-core s2-major outputs.
"""

import json

import numpy as np

import concourse.bass as bass
import concourse.mybir as mybir
from concourse.tile import TileContext
from concourse.bass import ts, ds
from concourse.bass_utils import run_bass_kernel_spmd

F32 = mybir.dt.float32
F32R = mybir.dt.float32r
AF = mybir.ActivationFunctionType

B, S, DM = 4, 2048, 1024
H, DK = 16, 64
HPC = 8          # heads per core (head-group size)
GD = HPC * DK    # 512, projected feature dim per core
S1C = 256        # s1 chunk width for the attention inner phase
NS1 = S // S1C   # 8
NS2 = S // 128   # 16 s2 chunks of 128
EG = 4           # s2 chunks whose exp is batched into one ACT instruction


class WaitSplitBass(bass.Bass):
    """This walrus build rejects instructions carrying more than one sem wait
    ("Too many sync wait commands"). Split extra waits into preceding
    same-engine NoOps at serialization time."""

    _ws_counter = 0

    def to_json_bytes(self) -> bytes:
        raw = super().to_json_bytes()
        d = json.loads(raw)
        changed = False
        for f in d.get("functions", []):
            for blk in f.get("blocks", []):
                out = []
                for inst in blk.get("instructions", []):
                    si = inst.get("sync_info")
                    waits = (si or {}).get("on_wait") or []
                    if len(waits) > 1:
                        changed = True
                        for w in waits[:-1]:
                            WaitSplitBass._ws_counter += 1
                            out.append({
                                "debug": inst.get("debug"),
                                "engine": inst["engine"],
                                "ins": [],
                                "name": f"I-ws{WaitSplitBass._ws_counter}",
                                "opcode": "NoOp",
                                "outs": [],
                                "sync_info": {"on_update": [], "on_wait": [w]},
                            })
                        si["on_wait"] = waits[-1:]
                    out.append(inst)
                blk["instructions"] = out
        if not changed:
            return raw
        return json.dumps(d, separators=(",", ":")).encode()


def _round_f32r(x: np.ndarray) -> np.ndarray:
    """Round fp32 to the float32r grid (11 mantissa bits, round-to-nearest)."""
    x = np.ascontiguousarray(x, dtype=np.float32)
    b = x.view(np.uint32).astype(np.uint64)
    return (((b + 0x800) & 0xFFFFF000).astype(np.uint32)).view(np.float32)


def _build_kernel() -> bass.Bass:
    nc = WaitSplitBass()

    qt = nc.dram_tensor("qt", [DM, S], F32R, kind="ExternalInput")
    kt = nc.dram_tensor("kt", [DM, S], F32R, kind="ExternalInput")
    vt = nc.dram_tensor("vt", [DM, S], F32R, kind="ExternalInput")
    wqt = nc.dram_tensor("wqt", [DM, GD], F32R, kind="ExternalInput")
    wkt = nc.dram_tensor("wkt", [DM, GD], F32R, kind="ExternalInput")
    wvt = nc.dram_tensor("wvt", [DM, GD], F32R, kind="ExternalInput")
    wot = nc.dram_tensor("wot", [GD, DM], F32R, kind="ExternalInput")
    bqs = nc.dram_tensor("bqs", [128, 4], F32, kind="ExternalInput")   # bq striped
    bks = nc.dram_tensor("bks", [128, 4], F32, kind="ExternalInput")   # bk striped
    bvr = nc.dram_tensor("bvr", [128, GD], F32, kind="ExternalInput")  # bv replicated

    attnt = nc.dram_tensor("attnt", [HPC, S, S], F32, kind="ExternalOutput")
    outt = nc.dram_tensor("outt", [DM, S], F32, kind="ExternalOutput")

    with TileContext(nc) as tc:
        with tc.tile_pool(name="pers", bufs=1) as pers:
            qt_sb = pers.tile([128, 4, S], F32R, tag="qt_sb")
            kt_sb = pers.tile([128, 4, S], F32R, tag="kt_sb")
            v_sb = pers.tile([128, NS2, HPC, DK + 1], F32R, tag="v_sb")
            ctxt_sb = pers.tile([128, 4, S], F32R, tag="ctxt_sb")
            bq_sb = pers.tile([128, 4], F32, tag="bq_sb")
            bk_sb = pers.tile([128, 4], F32, tag="bk_sb")
            bv_sb = pers.tile([128, GD], F32, tag="bv_sb")
            ones_f = pers.tile([128, 1], F32, tag="ones_f")
            ones_r = pers.tile([1, 128], F32R, tag="ones_r")

            nc.sync.dma_start(bq_sb[:], bqs[:])
            nc.sync.dma_start(bk_sb[:], bks[:])
            nc.sync.dma_start(bv_sb[:], bvr[:])
            nc.vector.memset(ones_f[:], 1.0)
            nc.vector.tensor_copy(ones_r[:], ones_f[0:1, 0:1].to_broadcast((1, 128)))
            # ones column of v (the column-sum piggyback)
            nc.vector.tensor_copy(
                v_sb[:, :, :, DK:DK + 1],
                ones_f[:, 0:1].to_broadcast((128, NS2, HPC, 1)),
            )

            # ---------------- projections ----------------
            with tc.tile_pool(name="wpool", bufs=2) as wpool, \
                 tc.tile_pool(name="xpool", bufs=3) as xpool, \
                 tc.tile_pool(name="ppsum", bufs=2, space="PSUM") as ppsum:

                for which, (x_dram, w_dram) in enumerate(
                    [(qt, wqt), (kt, wkt), (vt, wvt)]
                ):
                    w_sb = wpool.tile([128, 8, GD], F32R, tag="w_sb")
                    nc.sync.dma_start(
                        w_sb[:], w_dram.rearrange("(c p) j -> p c j", p=128)
                    )
                    for sblk in range(8):  # 256-wide chunks of S
                        xt = xpool.tile([128, 8, 256], F32R, tag="xt")
                        nc.sync.dma_start(
                            xt[:],
                            x_dram.rearrange("(c p) s -> p c s", p=128)[
                                :, :, ts(sblk, 256)
                            ],
                        )
                        if which < 2:
                            # qT / kT: out rows = (h,d), free = s
                            dest = qt_sb if which == 0 else kt_sb
                            bias = bq_sb if which == 0 else bk_sb
                            scale = 0.125 if which == 0 else 1.0
                            for m in range(4):
                                ps = ppsum.tile([128, 256], F32, tag="pps")
                                for c in range(8):
                                    nc.tensor.matmul(
                                        ps[:],
                                        w_sb[:, c, ts(m, 128)],
                                        xt[:, c, :],
                                        start=(c == 0),
                                        stop=(c == 7),
                                    )
                                nc.vector.tensor_scalar(
                                    dest[:, m, ts(sblk, 256)],
                                    ps[:],
                                    bias[:, m:m + 1],
                                    scale,
                                    mybir.AluOpType.add,
                                    mybir.AluOpType.mult,
                                )
                        else:
                            # v: out rows = s, free = (h,d)
                            for mi in range(2):
                                ps = ppsum.tile([128, GD], F32, tag="ppsv")
                                for c in range(8):
                                    nc.tensor.matmul(
                                        ps[:],
                                        xt[:, c, ts(mi, 128)],
                                        w_sb[:, c, :],
                                        start=(c == 0),
                                        stop=(c == 7),
                                    )
                                nc.vector.tensor_tensor(
                                    v_sb[:, sblk * 2 + mi, :, 0:DK],
                                    ps[:].rearrange("p (h d) -> p h d", d=DK),
                                    bv_sb[:].rearrange("p (h d) -> p h d", d=DK),
                                    mybir.AluOpType.add,
                                )

            # ---------------- attention ----------------
            with tc.tile_pool(name="epool", bufs=2) as epool, \
                 tc.tile_pool(name="npool", bufs=2) as npool, \
                 tc.tile_pool(name="spsum", bufs=2, space="PSUM") as spsum, \
                 tc.tile_pool(name="avpsum", bufs=2, space="PSUM") as avpsum, \
                 tc.tile_pool(name="rpsum", bufs=2, space="PSUM") as rpsum:

                attnt_r = attnt.rearrange("h (c p) s -> h p c s", p=128)
                for h in range(HPC):
                    hl = (h % 2) * DK       # partition offset of this head
                    hb = h // 2             # free-dim block of this head
                    for s1c in range(NS1):
                        s1 = ds(s1c * S1C, S1C)
                        expt = epool.tile([128, NS2, S1C], F32R, tag="expt")
                        avps = avpsum.tile([128, S1C], F32, tag="avps")
                        pend = []  # AV matmuls deferred one step (SW pipeline)
                        for g in range(NS2 // EG):
                            sps = spsum.tile([128, EG, S1C], F32, tag="sps")
                            for j in range(EG):
                                s2c = EG * g + j
                                nc.tensor.matmul(
                                    sps[:, j, :],
                                    kt_sb[ds(hl, DK), hb, ts(s2c, 128)],
                                    qt_sb[ds(hl, DK), hb, s1],
                                    start=True,
                                    stop=True,
                                )
                            for fn in pend:
                                fn()
                            pend = []
                            nc.scalar.activation(
                                expt[:, ds(EG * g, EG), :], sps[:], AF.Exp
                            )
                            for j in range(EG):
                                s2c = EG * g + j

                                def av(s2c=s2c):
                                    nc.tensor.matmul(
                                        avps[0:DK + 1, :],
                                        v_sb[:, s2c, h, :],
                                        expt[:, s2c, :],
                                        start=(s2c == 0),
                                        stop=(s2c == NS2 - 1),
                                        skip_group_check=True,
                                    )

                                pend.append(av)
                        for fn in pend:
                            fn()
                        # softmax denominator -> broadcast across partitions
                        recip_f = npool.tile([1, S1C], F32, tag="recip_f")
                        nc.vector.reciprocal(recip_f[:], avps[DK:DK + 1, :])
                        recip_r = npool.tile([1, S1C], F32R, tag="recip_r")
                        nc.vector.tensor_copy(recip_r[:], recip_f[:])
                        rps = rpsum.tile([128, S1C], F32, tag="rps")
                        nc.tensor.matmul(
                            rps[:], ones_r[:], recip_r[:], start=True, stop=True
                        )
                        rep = npool.tile([128, S1C], F32, tag="rep")
                        nc.vector.tensor_copy(rep[:], rps[:])
                        # normalize expT in place, write attn chunk
                        nc.vector.tensor_tensor(
                            expt[:],
                            expt[:],
                            rep[:, None, :].to_broadcast((128, NS2, S1C)),
                            mybir.AluOpType.mult,
                        )
                        nc.sync.dma_start(
                            attnt_r[h, :, :, s1], expt[:].bitcast(F32)
                        )
                        # normalized ctxT slice for this head
                        nc.vector.tensor_tensor(
                            ctxt_sb[ds(hl, DK), hb, s1],
                            avps[0:DK, :],
                            rep[0:DK, :],
                            mybir.AluOpType.mult,
                        )

            # ---------------- output projection ----------------
            with tc.tile_pool(name="wopool", bufs=1) as wopool, \
                 tc.tile_pool(name="ostage", bufs=3) as ostage, \
                 tc.tile_pool(name="opsum", bufs=2, space="PSUM") as opsum:
                wo_sb = wopool.tile([128, 4, DM], F32R, tag="wo_sb")
                nc.sync.dma_start(
                    wo_sb[:], wot.rearrange("(c p) e -> p c e", p=128)
                )
                for e in range(8):
                    for sc in range(4):
                        ps = opsum.tile([128, 512], F32, tag="ops")
                        for hd in range(4):
                            nc.tensor.matmul(
                                ps[:],
                                wo_sb[:, hd, ts(e, 128)],
                                ctxt_sb[:, hd, ts(sc, 512)],
                                start=(hd == 0),
                                stop=(hd == 3),
                            )
                        st = ostage.tile([128, 512], F32, tag="ost")
                        nc.vector.tensor_copy(st[:], ps[:])
                        nc.sync.dma_start(outt[ts(e, 128), ts(sc, 512)], st[:])

    return nc


_NC_CACHE = None


def get_nc():
    global _NC_CACHE
    if _NC_CACHE is None:
        _NC_CACHE = _build_kernel()
    return _NC_CACHE


def prepare_in_maps(Q, K, V, Wq, bq, Wk, bk, Wv, bv, Wo, bo):
    qts = [_round_f32r(Q[b].T) for b in range(B)]
    kts = [_round_f32r(K[b].T) for b in range(B)]
    vts = [_round_f32r(V[b].T) for b in range(B)]

    in_maps = []
    for core in range(8):
        b, hg = core // 2, core % 2
        sl = slice(hg * GD, (hg + 1) * GD)
        in_maps.append({
            "qt": qts[b],
            "kt": kts[b],
            "vt": vts[b],
            "wqt": _round_f32r(Wq[sl, :].T),
            "wkt": _round_f32r(Wk[sl, :].T),
            "wvt": _round_f32r(Wv[sl, :].T),
            "wot": _round_f32r(Wo[:, sl].T),
            "bqs": np.ascontiguousarray(bq[sl].reshape(4, 128).T),
            "bks": np.ascontiguousarray(bk[sl].reshape(4, 128).T),
            "bvr": np.ascontiguousarray(
                np.broadcast_to(bv[sl][None, :], (128, GD))
            ),
        })
    return in_maps


def kernel(Q, K, V, Wq, bq, Wk, bk, Wv, bv, Wo, bo):
    Q = np.asarray(Q, np.float32)
    K = np.asarray(K, np.float32)
    V = np.asarray(V, np.float32)
    Wq = np.asarray(Wq, np.float32)
    Wk = np.asarray(Wk, np.float32)
    Wv = np.asarray(Wv, np.float32)
    Wo = np.asarray(Wo, np.float32)
    bq = np.asarray(bq, np.float32)
    bk = np.asarray(bk, np.float32)
    bv = np.asarray(bv, np.float32)
    bo = np.asarray(bo, np.float32)

    nc = get_nc()
    in_maps = prepare_in_maps(Q, K, V, Wq, bq, Wk, bk, Wv, bv, Wo, bo)

    res = run_bass_kernel_spmd(nc, in_maps, core_ids=list(range(8)))
    kernel.last_result = res

    out = np.empty((B, S, DM), np.float32)
    for b in range(B):
        acc = res.results[2 * b]["outt"] + res.results[2 * b + 1]["outt"]
        out[b] = acc.T
    out += bo[None, None, :]

    # attn: per-core attnt is [h_local, s2, s1]; stack and view-transpose
    stacked = np.stack([res.results[c]["attnt"] for c in range(8)])
    attn = stacked.reshape(B, H, S, S).transpose(0, 1, 3, 2)
    return out, attn


# revision 8
# speedup vs baseline: 30.7027x; 30.7027x over previous
"""Multi-head attention (nn_MultiHeadAttention_67207648248315) on 8 TRN2 NeuronCores.

Sharding: core i handles batch b = i//2 and head-group hg = i%2 (8 of 16 heads).
Per core, everything is computed in a transposed layout (feature/key dim on
SBUF partitions) so the PE array contracts along partitions without any
on-chip activation transposes:

  qT/kT = W{q,k}_hg @ X_b.T          [512=(h,d), 2048=s]   (f32r matmuls)
  v     = V_b @ Wv_hg.T (+ ones col) [2048=s, 8*(64+1)]
  scoresT_h = kT_h.T-chunks @ qT_h   [s2, s1] tiles, softmax across partitions:
     exp on ACT -> expT; AV matmul with [v_h | 1] piggybacks the column sums;
     reciprocal broadcast across partitions via a rank-1 ones (x) recip matmul;
     normalized expT is the attention output (written s2-major, host views it
     back as [s1, s2]).
  outT_partial = Wo_hg-slice.T @ ctxT  [1024=e, 2048=s]

Host: pre-transposes/pre-rounds inputs (float32r = fp32 with 11 mantissa
bits), sums the two head-group partials for `out`, adds biases bo, and
returns `attn` as a transposed view of the per-core s2-major outputs.
"""

import json

import numpy as np

import concourse.bass as bass
import concourse.mybir as mybir
from concourse.tile import TileContext
from concourse.bass import ts, ds
from concourse.bass_utils import run_bass_kernel_spmd

F32 = mybir.dt.float32
F32R = mybir.dt.float32r
AF = mybir.ActivationFunctionType

B, S, DM = 4, 2048, 1024
H, DK = 16, 64
HPC = 8          # heads per core (head-group size)
GD = HPC * DK    # 512, projected feature dim per core
S1C = 256        # s1 chunk width for the attention inner phase
NS1 = S // S1C   # 8
NS2 = S // 128   # 16 s2 chunks of 128
EG = 4           # s2 chunks whose exp is batched into one ACT instruction


class WaitSplitBass(bass.Bass):
    """This walrus build rejects instructions carrying more than one sem wait
    ("Too many sync wait commands"). Split extra waits into preceding
    same-engine NoOps at serialization time."""

    _ws_counter = 0

    def to_json_bytes(self) -> bytes:
        raw = super().to_json_bytes()
        d = json.loads(raw)
        changed = False
        for f in d.get("functions", []):
            for blk in f.get("blocks", []):
                out = []
                for inst in blk.get("instructions", []):
                    si = inst.get("sync_info")
                    waits = (si or {}).get("on_wait") or []
                    if len(waits) > 1:
                        changed = True
                        for w in waits[:-1]:
                            WaitSplitBass._ws_counter += 1
                            out.append({
                                "debug": inst.get("debug"),
                                "engine": inst["engine"],
                                "ins": [],
                                "name": f"I-ws{WaitSplitBass._ws_counter}",
                                "opcode": "NoOp",
                                "outs": [],
                                "sync_info": {"on_update": [], "on_wait": [w]},
                            })
                        si["on_wait"] = waits[-1:]
                    out.append(inst)
                blk["instructions"] = out
        if not changed:
            return raw
        return json.dumps(d, separators=(",", ":")).encode()


def _round_f32r(x: np.ndarray) -> np.ndarray:
    """Round fp32 to the float32r grid (11 mantissa bits, round-to-nearest)."""
    x = np.ascontiguousarray(x, dtype=np.float32)
    b = x.view(np.uint32).astype(np.uint64)
    return (((b + 0x800) & 0xFFFFF000).astype(np.uint32)).view(np.float32)


def _build_kernel(repeat: int = 1) -> bass.Bass:
    nc = WaitSplitBass()

    qt = nc.dram_tensor("qt", [DM, S], F32R, kind="ExternalInput")
    kt = nc.dram_tensor("kt", [DM, S], F32R, kind="ExternalInput")
    vt = nc.dram_tensor("vt", [DM, S], F32R, kind="ExternalInput")
    wqt = nc.dram_tensor("wqt", [DM, GD], F32R, kind="ExternalInput")
    wkt = nc.dram_tensor("wkt", [DM, GD], F32R, kind="ExternalInput")
    wvt = nc.dram_tensor("wvt", [DM, GD], F32R, kind="ExternalInput")
    wot = nc.dram_tensor("wot", [GD, DM], F32R, kind="ExternalInput")
    bqs = nc.dram_tensor("bqs", [128, 4], F32, kind="ExternalInput")   # bq striped
    bks = nc.dram_tensor("bks", [128, 4], F32, kind="ExternalInput")   # bk striped
    bvr = nc.dram_tensor("bvr", [128, GD], F32, kind="ExternalInput")  # bv replicated

    attnt = nc.dram_tensor("attnt", [HPC, S, S], F32, kind="ExternalOutput")
    outt = nc.dram_tensor("outt", [DM, S], F32, kind="ExternalOutput")

    with TileContext(nc) as tc:
      for _rep in range(repeat):
        with tc.tile_pool(name="pers", bufs=1) as pers:
            qt_sb = pers.tile([128, 4, S], F32R, tag="qt_sb")
            kt_sb = pers.tile([128, 4, S], F32R, tag="kt_sb")
            v_sb = pers.tile([128, NS2, HPC, DK + 1], F32R, tag="v_sb")
            ctxt_sb = pers.tile([128, 4, S], F32R, tag="ctxt_sb")
            bq_sb = pers.tile([128, 4], F32, tag="bq_sb")
            bk_sb = pers.tile([128, 4], F32, tag="bk_sb")
            bv_sb = pers.tile([128, GD], F32, tag="bv_sb")
            ones_f = pers.tile([128, 1], F32, tag="ones_f")
            ones_r = pers.tile([1, 128], F32R, tag="ones_r")

            nc.sync.dma_start(bq_sb[:], bqs[:])
            nc.sync.dma_start(bk_sb[:], bks[:])
            nc.sync.dma_start(bv_sb[:], bvr[:])
            nc.vector.memset(ones_f[:], 1.0)
            nc.vector.tensor_copy(ones_r[:], ones_f[0:1, 0:1].to_broadcast((1, 128)))
            # ones column of v (the column-sum piggyback)
            nc.vector.tensor_copy(
                v_sb[:, :, :, DK:DK + 1],
                ones_f[:, 0:1].to_broadcast((128, NS2, HPC, 1)),
            )

            # ---------------- projections ----------------
            with tc.tile_pool(name="wpool", bufs=2) as wpool, \
                 tc.tile_pool(name="xpool", bufs=3) as xpool, \
                 tc.tile_pool(name="ppsum", bufs=2, space="PSUM") as ppsum:

                for which, (x_dram, w_dram) in enumerate(
                    [(qt, wqt), (kt, wkt), (vt, wvt)]
                ):
                    w_sb = wpool.tile([128, 8, GD], F32R, tag="w_sb")
                    nc.sync.dma_start(
                        w_sb[:], w_dram.rearrange("(c p) j -> p c j", p=128)
                    )
                    for sblk in range(8):  # 256-wide chunks of S
                        xt = xpool.tile([128, 8, 256], F32R, tag="xt")
                        nc.sync.dma_start(
                            xt[:],
                            x_dram.rearrange("(c p) s -> p c s", p=128)[
                                :, :, ts(sblk, 256)
                            ],
                        )
                        if which < 2:
                            # qT / kT: out rows = (h,d), free = s
                            dest = qt_sb if which == 0 else kt_sb
                            bias = bq_sb if which == 0 else bk_sb
                            scale = 0.125 if which == 0 else 1.0
                            for m in range(4):
                                ps = ppsum.tile([128, 256], F32, tag="pps")
                                for c in range(8):
                                    nc.tensor.matmul(
                                        ps[:],
                                        w_sb[:, c, ts(m, 128)],
                                        xt[:, c, :],
                                        start=(c == 0),
                                        stop=(c == 7),
                                    )
                                nc.vector.tensor_scalar(
                                    dest[:, m, ts(sblk, 256)],
                                    ps[:],
                                    bias[:, m:m + 1],
                                    scale,
                                    mybir.AluOpType.add,
                                    mybir.AluOpType.mult,
                                )
                        else:
                            # v: out rows = s, free = (h,d)
                            for mi in range(2):
                                ps = ppsum.tile([128, GD], F32, tag="ppsv")
                                for c in range(8):
                                    nc.tensor.matmul(
                                        ps[:],
                                        xt[:, c, ts(mi, 128)],
                                        w_sb[:, c, :],
                                        start=(c == 0),
                                        stop=(c == 7),
                                    )
                                nc.vector.tensor_tensor(
                                    v_sb[:, sblk * 2 + mi, :, 0:DK],
                                    ps[:].rearrange("p (h d) -> p h d", d=DK),
                                    bv_sb[:].rearrange("p (h d) -> p h d", d=DK),
                                    mybir.AluOpType.add,
                                )

            # ---------------- attention ----------------
            with tc.tile_pool(name="epool", bufs=2) as epool, \
                 tc.tile_pool(name="npool", bufs=2) as npool, \
                 tc.tile_pool(name="spsum", bufs=2, space="PSUM") as spsum, \
                 tc.tile_pool(name="avpsum", bufs=2, space="PSUM") as avpsum, \
                 tc.tile_pool(name="rpsum", bufs=2, space="PSUM") as rpsum:

                attnt_r = attnt.rearrange("h (c p) s -> h p c s", p=128)
                for h in range(HPC):
                    hl = (h % 2) * DK       # partition offset of this head
                    hb = h // 2             # free-dim block of this head
                    for s1c in range(NS1):
                        s1 = ds(s1c * S1C, S1C)
                        expt = epool.tile([128, NS2, S1C], F32R, tag="expt")
                        avps = avpsum.tile([128, S1C], F32, tag="avps")
                        pend = []  # AV matmuls deferred one step (SW pipeline)
                        for g in range(NS2 // EG):
                            sps = spsum.tile([128, EG, S1C], F32, tag="sps")
                            for j in range(EG):
                                s2c = EG * g + j
                                nc.tensor.matmul(
                                    sps[:, j, :],
                                    kt_sb[ds(hl, DK), hb, ts(s2c, 128)],
                                    qt_sb[ds(hl, DK), hb, s1],
                                    start=True,
                                    stop=True,
                                )
                            for fn in pend:
                                fn()
                            pend = []
                            nc.scalar.activation(
                                expt[:, ds(EG * g, EG), :], sps[:], AF.Exp
                            )
                            for j in range(EG):
                                s2c = EG * g + j

                                def av(s2c=s2c):
                                    nc.tensor.matmul(
                                        avps[0:DK + 1, :],
                                        v_sb[:, s2c, h, :],
                                        expt[:, s2c, :],
                                        start=(s2c == 0),
                                        stop=(s2c == NS2 - 1),
                                        skip_group_check=True,
                                    )

                                pend.append(av)
                        for fn in pend:
                            fn()
                        # softmax denominator -> broadcast across partitions
                        recip_f = npool.tile([1, S1C], F32, tag="recip_f")
                        nc.vector.reciprocal(recip_f[:], avps[DK:DK + 1, :])
                        recip_r = npool.tile([1, S1C], F32R, tag="recip_r")
                        nc.vector.tensor_copy(recip_r[:], recip_f[:])
                        rps = rpsum.tile([128, S1C], F32, tag="rps")
                        nc.tensor.matmul(
                            rps[:], ones_r[:], recip_r[:], start=True, stop=True
                        )
                        rep = npool.tile([128, S1C], F32, tag="rep")
                        nc.vector.tensor_copy(rep[:], rps[:])
                        # normalize expT in place, write attn chunk
                        nc.vector.tensor_tensor(
                            expt[:],
                            expt[:],
                            rep[:, None, :].to_broadcast((128, NS2, S1C)),
                            mybir.AluOpType.mult,
                        )
                        nc.sync.dma_start(
                            attnt_r[h, :, :, s1], expt[:].bitcast(F32)
                        )
                        # normalized ctxT slice for this head
                        nc.vector.tensor_tensor(
                            ctxt_sb[ds(hl, DK), hb, s1],
                            avps[0:DK, :],
                            rep[0:DK, :],
                            mybir.AluOpType.mult,
                        )

            # ---------------- output projection ----------------
            with tc.tile_pool(name="wopool", bufs=1) as wopool, \
                 tc.tile_pool(name="ostage", bufs=3) as ostage, \
                 tc.tile_pool(name="opsum", bufs=2, space="PSUM") as opsum:
                wo_sb = wopool.tile([128, 4, DM], F32R, tag="wo_sb")
                nc.sync.dma_start(
                    wo_sb[:], wot.rearrange("(c p) e -> p c e", p=128)
                )
                for e in range(8):
                    for sc in range(4):
                        ps = opsum.tile([128, 512], F32, tag="ops")
                        for hd in range(4):
                            nc.tensor.matmul(
                                ps[:],
                                wo_sb[:, hd, ts(e, 128)],
                                ctxt_sb[:, hd, ts(sc, 512)],
                                start=(hd == 0),
                                stop=(hd == 3),
                            )
                        st = ostage.tile([128, 512], F32, tag="ost")
                        nc.vector.tensor_copy(st[:], ps[:])
                        nc.sync.dma_start(outt[ts(e, 128), ts(sc, 512)], st[:])

    return nc


_NC_CACHE = {}


def get_nc(repeat: int = 1):
    if repeat not in _NC_CACHE:
        _NC_CACHE[repeat] = _build_kernel(repeat)
    return _NC_CACHE[repeat]


def prepare_in_maps(Q, K, V, Wq, bq, Wk, bk, Wv, bv, Wo, bo):
    qts = [_round_f32r(Q[b].T) for b in range(B)]
    kts = [_round_f32r(K[b].T) for b in range(B)]
    vts = [_round_f32r(V[b].T) for b in range(B)]

    in_maps = []
    for core in range(8):
        b, hg = core // 2, core % 2
        sl = slice(hg * GD, (hg + 1) * GD)
        in_maps.append({
            "qt": qts[b],
            "kt": kts[b],
            "vt": vts[b],
            "wqt": _round_f32r(Wq[sl, :].T),
            "wkt": _round_f32r(Wk[sl, :].T),
            "wvt": _round_f32r(Wv[sl, :].T),
            "wot": _round_f32r(Wo[:, sl].T),
            "bqs": np.ascontiguousarray(bq[sl].reshape(4, 128).T),
            "bks": np.ascontiguousarray(bk[sl].reshape(4, 128).T),
            "bvr": np.ascontiguousarray(
                np.broadcast_to(bv[sl][None, :], (128, GD))
            ),
        })
    return in_maps


def kernel(Q, K, V, Wq, bq, Wk, bk, Wv, bv, Wo, bo):
    Q = np.asarray(Q, np.float32)
    K = np.asarray(K, np.float32)
    V = np.asarray(V, np.float32)
    Wq = np.asarray(Wq, np.float32)
    Wk = np.asarray(Wk, np.float32)
    Wv = np.asarray(Wv, np.float32)
    Wo = np.asarray(Wo, np.float32)
    bq = np.asarray(bq, np.float32)
    bk = np.asarray(bk, np.float32)
    bv = np.asarray(bv, np.float32)
    bo = np.asarray(bo, np.float32)

    nc = get_nc()
    in_maps = prepare_in_maps(Q, K, V, Wq, bq, Wk, bk, Wv, bv, Wo, bo)

    res = run_bass_kernel_spmd(nc, in_maps, core_ids=list(range(8)))
    kernel.last_result = res

    out = np.empty((B, S, DM), np.float32)
    for b in range(B):
        acc = res.results[2 * b]["outt"] + res.results[2 * b + 1]["outt"]
        out[b] = acc.T
    out += bo[None, None, :]

    # attn: per-core attnt is [h_local, s2, s1]; stack and view-transpose
    stacked = np.stack([res.results[c]["attnt"] for c in range(8)])
    attn = stacked.reshape(B, H, S, S).transpose(0, 1, 3, 2)
    return out, attn
